# revision 9
# baseline (speedup 1.0000x reference)
"""MiMoV2 GQA attention (B=2, S=2048, HID=4096, 32 Q heads / 8 KV heads,
HD=128, VD=96, partial RoPE 64, causal) on 8 TRN2 NeuronCores.

Sharding: tensor-parallel over heads. Core c owns Q heads 4c..4c+3 and KV
head c (Wq/Wk/Wv column shards, Wo row shard). Activations replicated; the
row-parallel Wo partial outputs are summed on the host (the unshard step).

v2 over the 742us baseline (PE busy was 638us of 742us span):
  * startup: first weight/hidden chunks are the first DMAs on their
    queues (split per-chunk so MM0's deps land early); constants move to
    the vector queue, cos/sin + wo to the scalar queue.
  * phase B's score->exp->PV pipeline (depth-2 pends queue) now runs
    continuously across (b,qh) boundaries -- the per-qh drain stalls
    (~2.8us x 7) disappear; per-qh normalization (reciprocal chain +
    broadcast) hangs off pop-callbacks exactly as before, one qh late.
  * phase C for batch-0 token columns is interleaved into phase B's
    batch-1 pair stream (2 column-group matmul triplets per pair), so
    B's ACT-bound PE gaps are filled with Wo matmuls; PSUM rebalanced
    stps 4 + accb 2 + bcps 1 + cps 1 = 8 banks.
  * phase C for batch-1 columns runs in a fresh 8-bank scope after B.
  * PSUM drains spread across scalar/vector/gpsimd so no single engine
    paces the drain path.
"""
import numpy as np

import concourse.bacc as bacc
import concourse.mybir as mybir
import concourse.tile as tile

F32 = mybir.dt.float32
F32R = mybir.dt.float32r
BF16 = mybir.dt.bfloat16

B, S, HID = 2, 2048, 4096
NH, NKV, HD, VD = 32, 8, 128, 96
ROPE = 64
NCORES = 8
QH = NH // NCORES            # 4 q heads per core
T = B * S                    # 4096 tokens
OSH = QH * VD                # 384 output dims per core
CO = NH * VD                 # 3072 full output dim
THETA = 1000000.0
SCALE = float(HD ** -0.5)
NVB = S // 128               # 16 v blocks per batch
NCT = CO // 128              # 24 output row blocks

AFT = mybir.ActivationFunctionType


def _build():
    nc = bacc.Bacc("TRN2", target_bir_lowering=False, debug=False,
                   num_devices=NCORES)
    hiddent = nc.declare_dram_parameter("hiddent", [HID, T], BF16, False)
    wq = nc.declare_dram_parameter("wq", [HID, QH * HD], BF16, False)
    wk = nc.declare_dram_parameter("wk", [HID, HD], BF16, False)
    wv = nc.declare_dram_parameter("wv", [HID, VD], BF16, False)
    wo = nc.declare_dram_parameter("wo", [OSH, CO], BF16, False)
    cos = nc.declare_dram_parameter("cos", [ROPE, T], F32, False)
    sin = nc.declare_dram_parameter("sin", [ROPE, T], F32, False)
    tri = nc.declare_dram_parameter("tri", [128, 128], BF16, False)
    ident = nc.declare_dram_parameter("ident", [128, 128], BF16, False)
    out = nc.declare_dram_parameter("out", [CO, T], BF16, True)

    hT_r = hiddent.rearrange("(c p) t -> p c t", p=128)
    wq_r = wq.rearrange("(c p) o -> p c o", p=128)
    wk_r = wk.rearrange("(c p) o -> p c o", p=128)
    wv_r = wv.rearrange("(c p) o -> p c o", p=128)
    wo_r = wo.rearrange("(c p) o -> p c o", p=128)

    # at output rows qh*96..qh*96+96 split into [128,·] chunk segments:
    # (chunk, part_lo, part_hi, vd_lo, vd_hi) per qh.  Every source and
    # destination partition window must start at a multiple of 32 and not
    # cross its natural alignment block (64@32 is illegal, 32@32 is fine).
    AT_SEGS = {
        0: [(0, 0, 96, 0, 96)],
        1: [(0, 96, 128, 0, 32), (1, 0, 32, 32, 64), (1, 32, 64, 64, 96)],
        2: [(1, 64, 128, 0, 64), (2, 0, 32, 64, 96)],
        3: [(2, 32, 64, 0, 32), (2, 64, 96, 32, 64), (2, 96, 128, 64, 96)],
    }

    with tile.TileContext(nc) as tc:
        with (
            tc.tile_pool(name="cst", bufs=1) as cst,
            tc.tile_pool(name="stg", bufs=1) as stg,
        ):
            # persistent stages (SBUF-resident across phases)
            wo_sb = stg.tile([128, OSH // 128, CO], BF16, tag="wo")
            at_sb = stg.tile([128, OSH // 128, T], BF16, tag="atall")
            qs = [[stg.tile([128, S], BF16, tag=f"qs{b}_{h}",
                            name=f"qs{b}_{h}") for h in range(QH)]
                  for b in range(B)]
            ks = [stg.tile([128, S], BF16, tag=f"ks{b}", name=f"ks{b}")
                  for b in range(B)]
            vx = [stg.tile([128, NVB * (VD + 1)], BF16, tag=f"vx{b}",
                           name=f"vx{b}") for b in range(B)]

            # constants: all preamble work on the vector queue so the
            # sync/gpsimd queues start with phase A's critical DMAs
            id_sb = cst.tile([128, 128], BF16, tag="ident")
            msk_sb = cst.tile([128, 128], BF16, tag="msk")
            ones96_f = cst.tile([1, 96], F32, tag="ones96_f")
            ones96 = cst.tile([1, 96], F32R, tag="ones96")

            def emit_consts():
                nc.scalar.dma_start(out=id_sb[:], in_=ident[:])
                nc.scalar.dma_start(out=msk_sb[:], in_=tri[:])
                nc.vector.memset(ones96_f[:], 1.0)
                nc.vector.tensor_copy(ones96[:], ones96_f[:])
                for b in range(B):
                    nc.vector.memset(vx[b][:], 1.0)  # ones cols survive

            # ---------------- Phase A: QKV projections + RoPE ------------
            with (
                tc.tile_pool(name="wpool", bufs=1) as wpool,
                tc.tile_pool(name="hpool", bufs=5) as hpool,
                tc.tile_pool(name="rpool", bufs=3) as rpool,
                tc.tile_pool(name="apsum", bufs=1, space="PSUM") as apsum,
                tc.tile_pool(name="vtps", bufs=2, space="PSUM") as vtps,
            ):
                cos_sb = wpool.tile([ROPE, T], F32, tag="cos")
                sin_sb = wpool.tile([ROPE, T], F32, tag="sin")
                wq_sb = wpool.tile([128, HID // 128, QH * HD], BF16,
                                   tag="wq")
                wk_sb = wpool.tile([128, HID // 128, HD], BF16, tag="wk")
                wv_sb = wpool.tile([128, HID // 128, VD], BF16, tag="wv")
                wg_loaded = 0

                def load_weight_group(g):
                    gsl = slice(g * 4, (g + 1) * 4)
                    weng = nc.gpsimd if g % 2 == 0 else nc.sync
                    if g == 0:
                        # chunk 0 first so MM0's stationaries land early
                        for w_sb, w_r in ((wk_sb, wk_r), (wv_sb, wv_r),
                                          (wq_sb, wq_r)):
                            weng.dma_start(out=w_sb[:, 0:1, :],
                                           in_=w_r[:, 0:1, :])
                        for w_sb, w_r in ((wk_sb, wk_r), (wv_sb, wv_r),
                                          (wq_sb, wq_r)):
                            weng.dma_start(out=w_sb[:, 1:4, :],
                                           in_=w_r[:, 1:4, :])
                    else:
                        weng.dma_start(out=wk_sb[:, gsl, :],
                                       in_=wk_r[:, gsl, :])
                        weng.dma_start(out=wv_sb[:, gsl, :],
                                       in_=wv_r[:, gsl, :])
                        weng.dma_start(out=wq_sb[:, gsl, :],
                                       in_=wq_r[:, gsl, :])

                def rope_finish(src, dst, stsl, tsl, nm):
                    # dst[0:64] = src[0:64]*cos + rot(src)*sin ; rest cast
                    t1 = rpool.tile([ROPE, 512], F32, tag="t1",
                                    name=f"t1_{nm}")
                    nc.vector.tensor_mul(t1[0:32, :], src[32:64, :],
                                         sin_sb[32:64, tsl])
                    nc.vector.tensor_mul(t1[32:64, :], src[0:32, :],
                                         sin_sb[0:32, tsl])
                    qc = rpool.tile([ROPE, 512], F32, tag="qc",
                                    name=f"qc_{nm}")
                    nc.vector.tensor_mul(qc[:], src[0:ROPE, :],
                                         cos_sb[:, tsl])
                    nc.vector.tensor_add(dst[0:ROPE, stsl], qc[:], t1[:])
                    nc.scalar.copy(dst[ROPE:128, stsl], src[ROPE:128, :])

                for tt in range(T // 512):
                    b_, st4 = tt // 4, tt % 4
                    stsl = slice(st4 * 512, st4 * 512 + 512)
                    tsl = slice(tt * 512, (tt + 1) * 512)
                    qacc = [apsum.tile([128, 512], F32, tag=f"qacc{h}",
                                       name=f"qacc{h}_{tt}")
                            for h in range(QH)]
                    kacc = apsum.tile([128, 512], F32, tag="kacc",
                                      name=f"kacc_{tt}")
                    vacc = apsum.tile([VD, 512], F32, tag="vacc",
                                      name=f"vacc_{tt}")
                    for g in range(8):
                        ld = hpool.tile([128, 4, 512], BF16, tag="h",
                                        name=f"ld_{tt}_{g}")
                        heng = nc.sync if g % 2 == 0 else nc.gpsimd
                        if tt == 0 and g == 0:
                            # critical path: per-chunk loads, then the
                            # rest of the startup work
                            nc.sync.dma_start(out=ld[:, 0:1, :],
                                              in_=hT_r[:, 0:1, tsl])
                            load_weight_group(0)
                            wg_loaded = 1
                            nc.sync.dma_start(out=ld[:, 1:4, :],
                                              in_=hT_r[:, 1:4, tsl])
                            nc.scalar.dma_start(out=cos_sb[:], in_=cos[:])
                            nc.scalar.dma_start(out=sin_sb[:], in_=sin[:])
                            emit_consts()
                        else:
                            heng.dma_start(
                                out=ld[:], in_=hT_r[:, g * 4:(g + 1) * 4,
                                                    tsl])
                        while wg_loaded < min(8, g + 2 + 6 * tt):
                            load_weight_group(wg_loaded)
                            wg_loaded += 1
                        if tt == 2 and g == 0:
                            nc.scalar.dma_start(out=wo_sb[:], in_=wo_r[:])
                        for c4 in range(4):
                            hc = g * 4 + c4
                            st_, sp_ = hc == 0, hc == HID // 128 - 1
                            rhs = ld[:, c4, :]
                            nc.tensor.matmul(kacc[:], wk_sb[:, hc, :], rhs,
                                             start=st_, stop=sp_)
                            nc.tensor.matmul(vacc[:], wv_sb[:, hc, :], rhs,
                                             start=st_, stop=sp_)
                            for h in range(QH):
                                nc.tensor.matmul(
                                    qacc[h][:],
                                    wq_sb[:, hc, h * 128:(h + 1) * 128],
                                    rhs, start=st_, stop=sp_)
                    # boundary: K first (phase B consumes it first), then V
                    rope_finish(kacc, ks[b_], stsl, tsl, f"k{tt}")
                    vts = rpool.tile([VD, 512], BF16, tag="vts",
                                     name=f"vts_{tt}")
                    nc.scalar.copy(vts[:], vacc[:])
                    for h in range(QH):
                        rope_finish(qacc[h], qs[b_][h], stsl, tsl,
                                    f"q{tt}_{h}")
                    for sub in range(4):
                        vtp = vtps.tile([128, VD], BF16, tag="vtp",
                                        name=f"vtp_{tt}_{sub}")
                        nc.tensor.matmul(vtp[:],
                                         vts[:, sub * 128:(sub + 1) * 128],
                                         id_sb[0:VD, 0:VD], start=True,
                                         stop=True, is_transpose=True)
                        blk = st4 * 4 + sub
                        nc.vector.tensor_copy(
                            vx[b_][:, blk * (VD + 1):blk * (VD + 1) + VD],
                            vtp[:])

            # ------- Phase B: causal attention (+ phase C for batch-0
            # token columns interleaved into batch-1's pair stream) ------
            with (
                tc.tile_pool(name="bpool", bufs=2) as bpool,
                tc.tile_pool(name="ptpool", bufs=4) as ptpool,
                tc.tile_pool(name="stps", bufs=2, space="PSUM") as stps,
                tc.tile_pool(name="accb", bufs=2, space="PSUM") as accb,
                # bcp broadcasts and phase-C column groups share one
                # 2-deep [128,512] ring (4+2+2 = 8 PSUM banks total);
                # ring distance 2 keeps WAR waits off the PE queue
                tc.tile_pool(name="xps", bufs=2, space="PSUM") as xps,
                tc.tile_pool(name="opool", bufs=2) as opool,
            ):
                # -------- phase C machinery (batch-0 half) --------
                osbs = {}

                def make_citem(ct, tb):
                    def go():
                        cp = xps.tile([128, 512], F32, tag="x",
                                      name=f"cp_{ct}_{tb}")
                        for oc in range(OSH // 128):
                            nc.tensor.matmul(
                                cp[:],
                                wo_sb[:, oc, ct * 128:(ct + 1) * 128],
                                at_sb[:, oc, tb * 512:(tb + 1) * 512],
                                start=(oc == 0), stop=(oc == OSH // 128 - 1))
                        if tb == 0:
                            osbs[ct] = opool.tile([128, S], BF16,
                                                  tag="osb",
                                                  name=f"osb0_{ct}")
                        osb = osbs[ct]
                        dsl = slice(tb * 512, tb * 512 + 512)
                        if (ct * 4 + tb) % 2 == 0:
                            nc.scalar.copy(osb[:, dsl], cp[:])
                        else:
                            nc.vector.tensor_copy(osb[:, dsl], cp[:])
                        if tb == 3:
                            rsl = slice(ct * 128, (ct + 1) * 128)
                            nc.sync.dma_start(out=out[rsl, 0:1024],
                                              in_=osb[:, 0:1024])
                            nc.gpsimd.dma_start(out=out[rsl, 1024:2048],
                                                in_=osb[:, 1024:2048])
                    return go

                citems = [make_citem(ct, tb)
                          for ct in range(NCT) for tb in range(4)]
                citems.reverse()          # pop() from the front via pop()

                def pop_citem():
                    if citems:
                        citems.pop()()

                # -------- phase B machinery --------
                def emit_pv(p):
                    acc_, pt2_, jp_, ib_, vxb_, last_ = p[:6]
                    first = jp_ == 0
                    if jp_ >= 2 * ib_:          # diagonal pair
                        s0 = 2 * (jp_ - 2 * ib_)
                        r0, r1 = s0 * 128, s0 * 128 + 128
                        j0 = 4 * ib_ + s0
                        nc.tensor.matmul(
                            acc_[:, r0:512],
                            vxb_[:, j0 * 97:j0 * 97 + 97],
                            pt2_[:, r0:512], start=first, stop=False)
                        nc.tensor.matmul(
                            acc_[:, r1:512],
                            vxb_[:, (j0 + 1) * 97:(j0 + 1) * 97 + 97],
                            pt2_[:, 512 + r1:1024], start=False, stop=last_)
                    else:
                        j0 = 2 * jp_
                        nc.tensor.matmul(acc_[:],
                                         vxb_[:, j0 * 97:j0 * 97 + 97],
                                         pt2_[:, 0:512], start=first,
                                         stop=False)
                        nc.tensor.matmul(
                            acc_[:],
                            vxb_[:, (j0 + 1) * 97:(j0 + 1) * 97 + 97],
                            pt2_[:, 512:1024], start=False, stop=last_)

                norm_tail = [None]

                def flush_norm():
                    if norm_tail[0] is None:
                        return
                    b_, qh_, atus_, rrs_ = norm_tail[0]
                    norm_tail[0] = None
                    for ib_ in range(4):
                        bcpt = xps.tile([128, 512], F32, tag="x",
                                        name=f"bcp_{b_}_{qh_}_{ib_}")
                        bcp = bcpt[0:VD, :]
                        nc.tensor.matmul(bcp[:], ones96[:], rrs_[ib_][:],
                                         start=True, stop=True)
                        csl = slice(b_ * S + ib_ * 512,
                                    b_ * S + (ib_ + 1) * 512)
                        for (c, pa, pb, va, vb_) in AT_SEGS[qh_]:
                            nc.vector.tensor_mul(at_sb[pa:pb, c, csl],
                                                 atus_[ib_][va:vb_, :],
                                                 bcp[va:vb_, :])

                pends = []

                def pop_pv():
                    p = pends.pop(0)
                    emit_pv(p)
                    acc_, _, jp_, ib_, _, last_, b_, qh_, st = p
                    if not last_:
                        return
                    # drain this q tile: denominator row + bf16 cast
                    dn4_, atus_ = st["dn4"], st["atus"]
                    nc.vector.tensor_copy(
                        dn4_[32 * ib_:32 * ib_ + 1, :],
                        acc_[VD:VD + 1, :])
                    atu = bpool.tile([VD, 512], BF16, tag="atu", bufs=8,
                                     name=f"atu_{b_}_{qh_}_{ib_}")
                    nc.vector.tensor_copy(atu[:], acc_[0:VD, :])
                    atus_.append(atu)
                    if ib_ == 3:
                        # qh complete: reciprocal chain + defer normalize
                        rec4 = bpool.tile([97, 512], F32, tag="rec4",
                                          name=f"rec4_{b_}_{qh_}")
                        nc.vector.reciprocal(rec4[:], dn4_[:])
                        rrs = []
                        for i4 in range(4):
                            rr = bpool.tile([1, 512], F32R, tag="rr",
                                            bufs=8,
                                            name=f"rr_{b_}_{qh_}_{i4}")
                            nc.vector.tensor_copy(
                                rr[:], rec4[32 * i4:32 * i4 + 1, :])
                            rrs.append(rr)
                        norm_tail[0] = (b_, qh_, atus_, rrs)

                for b in range(B):
                    ksb, vxb = ks[b], vx[b]
                    for qh in range(QH):
                        dn4 = bpool.tile([97, 512], F32, tag="dn4",
                                         name=f"dn4_{b}_{qh}")
                        st = {"dn4": dn4, "atus": []}
                        for ib in range(4):
                            qcols = qs[b][qh][:, ib * 512:(ib + 1) * 512]
                            acc = accb.tile([VD + 1, 512], F32, tag="acc",
                                            name=f"acc_{b}_{qh}_{ib}")
                            npair = 2 * ib + 2
                            for jp in range(npair):
                                stp2 = stps.tile([128, 1024], F32,
                                                 tag="stp")
                                pt2 = ptpool.tile([128, 1024], BF16,
                                                  tag="pt")
                                if jp >= 2 * ib:     # diagonal pair
                                    s0 = 2 * (jp - 2 * ib)
                                    r0, r1 = s0 * 128, s0 * 128 + 128
                                    j0 = 4 * ib + s0
                                    nc.tensor.matmul(
                                        stp2[:, r0:512],
                                        ksb[:, j0 * 128:(j0 + 1) * 128],
                                        qcols[:, r0:512], start=True,
                                        stop=True)
                                    nc.tensor.matmul(
                                        stp2[:, 512 + r1:1024],
                                        ksb[:, (j0 + 1) * 128:
                                            (j0 + 2) * 128],
                                        qcols[:, r1:512], start=True,
                                        stop=True)
                                    nc.scalar.activation(
                                        pt2[:, r0:512], stp2[:, r0:512],
                                        AFT.Exp, scale=SCALE)
                                    nc.scalar.activation(
                                        pt2[:, 512 + r1:1024],
                                        stp2[:, 512 + r1:1024],
                                        AFT.Exp, scale=SCALE)
                                    nc.vector.tensor_mul(
                                        pt2[:, r0:r0 + 128],
                                        pt2[:, r0:r0 + 128], msk_sb[:])
                                    nc.vector.tensor_mul(
                                        pt2[:, 512 + r1:512 + r1 + 128],
                                        pt2[:, 512 + r1:512 + r1 + 128],
                                        msk_sb[:])
                                else:                # two full blocks
                                    j0 = 2 * jp
                                    nc.tensor.matmul(
                                        stp2[:, 0:512],
                                        ksb[:, j0 * 128:(j0 + 1) * 128],
                                        qcols, start=True, stop=True)
                                    nc.tensor.matmul(
                                        stp2[:, 512:1024],
                                        ksb[:, (j0 + 1) * 128:
                                            (j0 + 2) * 128],
                                        qcols, start=True, stop=True)
                                    nc.scalar.activation(
                                        pt2[:], stp2[:], AFT.Exp,
                                        scale=SCALE)
                                if (ib == 2 and jp == 1) or \
                                        (b == 1 and qh == 0 and ib == 1
                                         and jp == 0):
                                    # previous qh's normalization tail,
                                    # emitted once its reciprocal chain
                                    # has cleared the DVE queue
                                    flush_norm()
                                pends.append(
                                    (acc, pt2, jp, ib, vxb,
                                     jp == npair - 1, b, qh, st))
                                while len(pends) > 2:
                                    pop_pv()
                                if b == 1 and qh >= 1:
                                    pop_citem()
                                    pop_citem()
                while pends:
                    pop_pv()
                flush_norm()              # b1 qh3
                while citems:             # any leftover b0 column groups
                    pop_citem()

            # ---------------- Phase C: batch-1 token columns -------------
            with (
                tc.tile_pool(name="opool2", bufs=2) as opool2,
                tc.tile_pool(name="tps", bufs=2, space="PSUM") as tps,
            ):
                for ct in range(NCT):
                    ops = [tps.tile([128, 512], F32, tag=f"c{tb}",
                                    name=f"ops{ct}_{tb}")
                           for tb in range(4)]
                    for oc in range(OSH // 128):
                        for tb in range(4):
                            nc.tensor.matmul(
                                ops[tb][:],
                                wo_sb[:, oc, ct * 128:(ct + 1) * 128],
                                at_sb[:, oc, S + tb * 512:S + (tb + 1) * 512],
                                start=(oc == 0), stop=(oc == OSH // 128 - 1))
                    osb = opool2.tile([128, S], BF16, tag="osb2",
                                      name=f"osb2_{ct}")
                    for tb in range(4):
                        tbs = slice(tb * 512, (tb + 1) * 512)
                        if tb % 2 == 0:
                            nc.vector.tensor_copy(osb[:, tbs], ops[tb][:])
                        else:
                            nc.scalar.copy(osb[:, tbs], ops[tb][:])
                    rsl = slice(ct * 128, (ct + 1) * 128)
                    nc.sync.dma_start(out=out[rsl, S:S + 1024],
                                      in_=osb[:, 0:1024])
                    nc.gpsimd.dma_start(out=out[rsl, S + 1024:S + 2048],
                                        in_=osb[:, 1024:2048])

    nc.compile()
    return nc


_NC_CACHE = None


def _get_nc():
    global _NC_CACHE
    if _NC_CACHE is None:
        _NC_CACHE = _build()
    return _NC_CACHE


def _host_tables(position_ids):
    pos = np.asarray(position_ids).reshape(-1)[:S].astype(np.float64)
    inv_freq = 1.0 / (THETA ** (np.arange(0, ROPE, 2, dtype=np.float64) / ROPE))
    freqs = np.outer(pos, inv_freq)                       # [S, ROPE/2]
    emb = np.concatenate([freqs, freqs], axis=-1)         # [S, ROPE]
    cos_t = np.tile(np.cos(emb).astype(np.float32).T, (1, B))  # [ROPE, T]
    sinf = np.sin(freqs).astype(np.float32).T                  # [ROPE/2, S]
    # rows 0:32 = +sin (t1[32:64] = q[0:32]*sin)
    # rows 32:64 = -sin (t1[0:32] = -q[32:64]*sin)
    sin_t = np.tile(np.concatenate([sinf, -sinf], axis=0), (1, B))
    return cos_t, sin_t


def kernel(hidden_states, attention_mask, position_ids, Wq, Wk, Wv, Wo,
           _trace=False, _tmpdir=None):
    import ml_dtypes
    from concourse.bass_utils import run_bass_kernel_spmd
    bf16 = ml_dtypes.bfloat16

    hidden = np.asarray(hidden_states, dtype=np.float32).reshape(T, HID)
    hiddent = np.ascontiguousarray(hidden.T).astype(bf16)
    Wq = np.asarray(Wq, dtype=np.float32)
    Wk = np.asarray(Wk, dtype=np.float32)
    Wv = np.asarray(Wv, dtype=np.float32)
    Wo = np.asarray(Wo, dtype=np.float32)
    cos_t, sin_t = _host_tables(position_ids)
    m = np.asarray(attention_mask).reshape(S, S)
    tri = np.ascontiguousarray((m[0:128, 0:128] == 0.0).T).astype(bf16)
    ident = np.eye(128, dtype=np.float32).astype(bf16)

    nc = _get_nc()
    in_maps = []
    for c in range(NCORES):
        in_maps.append({
            "hiddent": hiddent,
            "wq": np.ascontiguousarray(
                Wq[:, c * QH * HD:(c + 1) * QH * HD]).astype(bf16),
            "wk": np.ascontiguousarray(Wk[:, c * HD:(c + 1) * HD]).astype(bf16),
            "wv": np.ascontiguousarray(Wv[:, c * VD:(c + 1) * VD]).astype(bf16),
            "wo": np.ascontiguousarray(Wo[c * OSH:(c + 1) * OSH, :]).astype(bf16),
            "cos": cos_t, "sin": sin_t,
            "tri": tri, "ident": ident,
        })
    res = run_bass_kernel_spmd(nc, in_maps, list(range(NCORES)),
                               trace=_trace, tmpdir=_tmpdir)
    full = np.zeros((CO, T), dtype=np.float32)
    for c in range(NCORES):
        full += res.results[c]["out"].astype(np.float32)
    out = np.ascontiguousarray(full.T).reshape(B, S, CO)
    if _trace:
        kernel.last_exec_time_ns = res.exec_time_ns
        kernel.last_profile = res.profile_json
    return out


# revision 17
# speedup vs baseline: 1.0785x; 1.0785x over previous
"""MiMoV2 GQA attention (B=2, S=2048, HID=4096, 32 Q heads / 8 KV heads,
HD=128, VD=96, partial RoPE 64, causal) on 8 TRN2 NeuronCores.

Sharding: tensor-parallel over heads. Core c owns Q heads 4c..4c+3 and KV
head c (Wq/Wk/Wv column shards, Wo row shard). Activations replicated; the
row-parallel Wo partial outputs are summed on the host (the unshard step).

v2 over the 742us baseline (PE busy was 638us of 742us span):
  * startup: first weight/hidden chunks are the first DMAs on their
    queues (split per-chunk so MM0's deps land early); constants move to
    the vector queue, cos/sin + wo to the scalar queue.
  * phase B's score->exp->PV pipeline (depth-2 pends queue) now runs
    continuously across (b,qh) boundaries -- the per-qh drain stalls
    (~2.8us x 7) disappear; per-qh normalization (reciprocal chain +
    broadcast) hangs off pop-callbacks exactly as before, one qh late.
  * phase C for batch-0 token columns is interleaved into phase B's
    batch-1 pair stream (2 column-group matmul triplets per pair), so
    B's ACT-bound PE gaps are filled with Wo matmuls; PSUM rebalanced
    stps 4 + accb 2 + bcps 1 + cps 1 = 8 banks.
  * phase C for batch-1 columns runs in a fresh 8-bank scope after B.
  * PSUM drains spread across scalar/vector/gpsimd so no single engine
    paces the drain path.
"""
import numpy as np

import concourse.bacc as bacc
import concourse.mybir as mybir
import concourse.tile as tile

F32 = mybir.dt.float32
F32R = mybir.dt.float32r
BF16 = mybir.dt.bfloat16

B, S, HID = 2, 2048, 4096
NH, NKV, HD, VD = 32, 8, 128, 96
ROPE = 64
NCORES = 8
QH = NH // NCORES            # 4 q heads per core
T = B * S                    # 4096 tokens
OSH = QH * VD                # 384 output dims per core
CO = NH * VD                 # 3072 full output dim
THETA = 1000000.0
SCALE = float(HD ** -0.5)
NVB = S // 128               # 16 v blocks per batch
NCT = CO // 128              # 24 output row blocks

AFT = mybir.ActivationFunctionType


def _build():
    nc = bacc.Bacc("TRN2", target_bir_lowering=False, debug=False,
                   num_devices=NCORES)
    hiddent = nc.declare_dram_parameter("hiddent", [HID, T], BF16, False)
    wq = nc.declare_dram_parameter("wq", [HID, QH * HD], BF16, False)
    wk = nc.declare_dram_parameter("wk", [HID, HD], BF16, False)
    wv = nc.declare_dram_parameter("wv", [HID, VD], BF16, False)
    wo = nc.declare_dram_parameter("wo", [OSH, CO], BF16, False)
    cos = nc.declare_dram_parameter("cos", [ROPE, T], F32, False)
    sin = nc.declare_dram_parameter("sin", [ROPE, T], F32, False)
    tri = nc.declare_dram_parameter("tri", [128, 128], BF16, False)
    ident = nc.declare_dram_parameter("ident", [128, 128], BF16, False)
    out = nc.declare_dram_parameter("out", [CO, T], BF16, True)

    hT_r = hiddent.rearrange("(c p) t -> p c t", p=128)
    wq_r = wq.rearrange("(c p) o -> p c o", p=128)
    wk_r = wk.rearrange("(c p) o -> p c o", p=128)
    wv_r = wv.rearrange("(c p) o -> p c o", p=128)
    wo_r = wo.rearrange("(c p) o -> p c o", p=128)

    # at output rows qh*96..qh*96+96 split into [128,·] chunk segments:
    # (chunk, part_lo, part_hi, vd_lo, vd_hi) per qh.  Every source and
    # destination partition window must start at a multiple of 32 and not
    # cross its natural alignment block (64@32 is illegal, 32@32 is fine).
    AT_SEGS = {
        0: [(0, 0, 96, 0, 96)],
        1: [(0, 96, 128, 0, 32), (1, 0, 32, 32, 64), (1, 32, 64, 64, 96)],
        2: [(1, 64, 128, 0, 64), (2, 0, 32, 64, 96)],
        3: [(2, 32, 64, 0, 32), (2, 64, 96, 32, 64), (2, 96, 128, 64, 96)],
    }

    with tile.TileContext(nc) as tc:
        with (
            tc.tile_pool(name="cst", bufs=1) as cst,
            tc.tile_pool(name="stg", bufs=1) as stg,
        ):
            # persistent stages (SBUF-resident across phases)
            wo_sb = stg.tile([128, OSH // 128, CO], BF16, tag="wo")
            at_sb = stg.tile([128, OSH // 128, T], BF16, tag="atall")
            qs = [[stg.tile([128, S], BF16, tag=f"qs{b}_{h}",
                            name=f"qs{b}_{h}") for h in range(QH)]
                  for b in range(B)]
            ks = [stg.tile([128, S], BF16, tag=f"ks{b}", name=f"ks{b}")
                  for b in range(B)]
            vx = [stg.tile([128, NVB * (VD + 1)], BF16, tag=f"vx{b}",
                           name=f"vx{b}") for b in range(B)]

            # constants: all preamble work on the vector queue so the
            # sync/gpsimd queues start with phase A's critical DMAs
            id_sb = cst.tile([128, 128], BF16, tag="ident")
            msk_sb = cst.tile([128, 128], BF16, tag="msk")
            ones96_f = cst.tile([1, 96], F32, tag="ones96_f")
            ones96 = cst.tile([1, 96], F32R, tag="ones96")

            def emit_consts():
                nc.scalar.dma_start(out=id_sb[:], in_=ident[:])
                nc.scalar.dma_start(out=msk_sb[:], in_=tri[:])
                nc.vector.memset(ones96_f[:], 1.0)
                nc.vector.tensor_copy(ones96[:], ones96_f[:])
                for b in range(B):
                    nc.vector.memset(vx[b][:], 1.0)  # ones cols survive

            # ---------------- Phase A: QKV projections + RoPE ------------
            with (
                tc.tile_pool(name="wpool", bufs=1) as wpool,
                tc.tile_pool(name="hpool", bufs=5) as hpool,
                tc.tile_pool(name="rpool", bufs=3) as rpool,
                tc.tile_pool(name="apsum", bufs=1, space="PSUM") as apsum,
                tc.tile_pool(name="vtps", bufs=2, space="PSUM") as vtps,
            ):
                cos_sb = wpool.tile([ROPE, T], F32, tag="cos")
                sin_sb = wpool.tile([ROPE, T], F32, tag="sin")
                wq_sb = wpool.tile([128, HID // 128, QH * HD], BF16,
                                   tag="wq")
                wk_sb = wpool.tile([128, HID // 128, HD], BF16, tag="wk")
                wv_sb = wpool.tile([128, HID // 128, VD], BF16, tag="wv")
                wg_loaded = 0

                def load_weight_group(g):
                    gsl = slice(g * 4, (g + 1) * 4)
                    weng = nc.gpsimd if g % 2 == 0 else nc.sync
                    oeng = nc.sync if g % 2 == 0 else nc.gpsimd
                    if g == 0:
                        # chunk 0 spread over three queues so MM0's
                        # stationaries land as early as possible
                        nc.gpsimd.dma_start(out=wk_sb[:, 0:1, :],
                                            in_=wk_r[:, 0:1, :])
                        nc.gpsimd.dma_start(out=wv_sb[:, 0:1, :],
                                            in_=wv_r[:, 0:1, :])
                        nc.scalar.dma_start(out=wq_sb[:, 0:1, :],
                                            in_=wq_r[:, 0:1, :])
                        nc.gpsimd.dma_start(out=wk_sb[:, 1:4, :],
                                            in_=wk_r[:, 1:4, :])
                        nc.gpsimd.dma_start(out=wv_sb[:, 1:4, :],
                                            in_=wv_r[:, 1:4, :])
                        nc.scalar.dma_start(out=wq_sb[:, 1:4, :],
                                            in_=wq_r[:, 1:4, :])
                    else:
                        # wk+wv on one queue, the bigger wq on the other
                        weng.dma_start(out=wk_sb[:, gsl, :],
                                       in_=wk_r[:, gsl, :])
                        weng.dma_start(out=wv_sb[:, gsl, :],
                                       in_=wv_r[:, gsl, :])
                        oeng.dma_start(out=wq_sb[:, gsl, :],
                                       in_=wq_r[:, gsl, :])

                def rope_finish(src, dst, stsl, tsl, nm):
                    # dst[0:64] = src[0:64]*cos + rot(src)*sin ; rest cast
                    t1 = rpool.tile([ROPE, 512], F32, tag="t1",
                                    name=f"t1_{nm}")
                    nc.vector.tensor_mul(t1[0:32, :], src[32:64, :],
                                         sin_sb[32:64, tsl])
                    nc.vector.tensor_mul(t1[32:64, :], src[0:32, :],
                                         sin_sb[0:32, tsl])
                    qc = rpool.tile([ROPE, 512], F32, tag="qc",
                                    name=f"qc_{nm}")
                    nc.vector.tensor_mul(qc[:], src[0:ROPE, :],
                                         cos_sb[:, tsl])
                    # final add reads only SBUF: offload to the pool
                    # engine (gpsimd cannot touch PSUM but qc/t1 are SBUF)
                    nc.gpsimd.tensor_add(dst[0:ROPE, stsl], qc[:], t1[:])
                    nc.scalar.copy(dst[ROPE:128, stsl], src[ROPE:128, :])

                for tt in range(T // 512):
                    b_, st4 = tt // 4, tt % 4
                    stsl = slice(st4 * 512, st4 * 512 + 512)
                    tsl = slice(tt * 512, (tt + 1) * 512)
                    qacc = [apsum.tile([128, 512], F32, tag=f"qacc{h}",
                                       name=f"qacc{h}_{tt}")
                            for h in range(QH)]
                    kacc = apsum.tile([128, 512], F32, tag="kacc",
                                      name=f"kacc_{tt}")
                    vacc = apsum.tile([VD, 512], F32, tag="vacc",
                                      name=f"vacc_{tt}")
                    for g in range(8):
                        ld = hpool.tile([128, 4, 512], BF16, tag="h",
                                        name=f"ld_{tt}_{g}")
                        heng = nc.sync if g % 2 == 0 else nc.gpsimd
                        if tt == 0 and g == 0:
                            # critical path: per-chunk loads, then the
                            # rest of the startup work
                            nc.sync.dma_start(out=ld[:, 0:1, :],
                                              in_=hT_r[:, 0:1, tsl])
                            load_weight_group(0)
                            wg_loaded = 1
                            nc.sync.dma_start(out=ld[:, 1:4, :],
                                              in_=hT_r[:, 1:4, tsl])
                            nc.scalar.dma_start(out=cos_sb[:], in_=cos[:])
                            nc.scalar.dma_start(out=sin_sb[:], in_=sin[:])
                            emit_consts()
                        else:
                            heng.dma_start(
                                out=ld[:], in_=hT_r[:, g * 4:(g + 1) * 4,
                                                    tsl])
                        while wg_loaded < min(8, g + 3 + 6 * tt):
                            load_weight_group(wg_loaded)
                            wg_loaded += 1
                        if tt == 2 and g == 0:
                            nc.scalar.dma_start(out=wo_sb[:], in_=wo_r[:])
                        for c4 in range(4):
                            hc = g * 4 + c4
                            st_, sp_ = hc == 0, hc == HID // 128 - 1
                            rhs = ld[:, c4, :]
                            nc.tensor.matmul(kacc[:], wk_sb[:, hc, :], rhs,
                                             start=st_, stop=sp_)
                            nc.tensor.matmul(vacc[:], wv_sb[:, hc, :], rhs,
                                             start=st_, stop=sp_)
                            for h in range(QH):
                                nc.tensor.matmul(
                                    qacc[h][:],
                                    wq_sb[:, hc, h * 128:(h + 1) * 128],
                                    rhs, start=st_, stop=sp_)
                    # boundary: K first (the next tt's kacc matmul reuses
                    # its bank first).  At the LAST tt, drain in the order
                    # phase B's first allocations want the banks instead
                    # (stps buf0 = banks 0-1 = qacc0/1, accb buf0 = kacc).
                    horder = (0, 1, None, 2, 3) if tt == 7 \
                        else (None, 0, 1, 2, 3)
                    for h in horder:
                        if h is None:
                            rope_finish(kacc, ks[b_], stsl, tsl, f"k{tt}")
                            vts = rpool.tile([VD, 512], BF16, tag="vts",
                                             name=f"vts_{tt}")
                            nc.scalar.copy(vts[:], vacc[:])
                        else:
                            rope_finish(qacc[h], qs[b_][h], stsl, tsl,
                                        f"q{tt}_{h}")
                    for sub in range(4):
                        vtp = vtps.tile([128, VD], BF16, tag="vtp",
                                        name=f"vtp_{tt}_{sub}")
                        nc.tensor.matmul(vtp[:],
                                         vts[:, sub * 128:(sub + 1) * 128],
                                         id_sb[0:VD, 0:VD], start=True,
                                         stop=True, is_transpose=True)
                        blk = st4 * 4 + sub
                        nc.vector.tensor_copy(
                            vx[b_][:, blk * (VD + 1):blk * (VD + 1) + VD],
                            vtp[:])

            # ------- Phase B: causal attention (+ phase C for batch-0
            # token columns interleaved into batch-1's pair stream) ------
            with (
                tc.tile_pool(name="bpool", bufs=2) as bpool,
                tc.tile_pool(name="ptpool", bufs=4) as ptpool,
                tc.tile_pool(name="stps", bufs=2, space="PSUM") as stps,
                tc.tile_pool(name="accb", bufs=2, space="PSUM") as accb,
                # bcp broadcasts and phase-C column groups share one
                # 2-deep [128,512] ring (4+2+2 = 8 PSUM banks total);
                # ring distance 2 keeps WAR waits off the PE queue
                tc.tile_pool(name="xps", bufs=2, space="PSUM") as xps,
                tc.tile_pool(name="opool", bufs=2) as opool,
            ):
                # -------- phase C machinery (batch-0 half) --------
                osbs = {}

                def make_citem(ct, tb):
                    def go():
                        cp = xps.tile([128, 512], F32, tag="x",
                                      name=f"cp_{ct}_{tb}")
                        for oc in range(OSH // 128):
                            nc.tensor.matmul(
                                cp[:],
                                wo_sb[:, oc, ct * 128:(ct + 1) * 128],
                                at_sb[:, oc, tb * 512:(tb + 1) * 512],
                                start=(oc == 0), stop=(oc == OSH // 128 - 1))
                        if tb == 0:
                            osbs[ct] = opool.tile([128, S], BF16,
                                                  tag="osb",
                                                  name=f"osb0_{ct}")
                        osb = osbs[ct]
                        dsl = slice(tb * 512, tb * 512 + 512)
                        if (ct * 4 + tb) % 2 == 0:
                            nc.scalar.copy(osb[:, dsl], cp[:])
                        else:
                            nc.vector.tensor_copy(osb[:, dsl], cp[:])
                        if tb == 3:
                            rsl = slice(ct * 128, (ct + 1) * 128)
                            nc.sync.dma_start(out=out[rsl, 0:1024],
                                              in_=osb[:, 0:1024])
                            nc.gpsimd.dma_start(out=out[rsl, 1024:2048],
                                                in_=osb[:, 1024:2048])
                    return go

                citems = [make_citem(ct, tb)
                          for ct in range(NCT) for tb in range(4)]
                citems.reverse()          # pop() from the front via pop()

                def pop_citem():
                    if citems:
                        citems.pop()()

                # -------- phase B machinery --------
                def emit_pv(p):
                    acc_, pt2_, jp_, ib_, vxb_, last_ = p[:6]
                    first = jp_ == 0
                    if jp_ >= 2 * ib_:          # diagonal pair
                        s0 = 2 * (jp_ - 2 * ib_)
                        r0, r1 = s0 * 128, s0 * 128 + 128
                        j0 = 4 * ib_ + s0
                        nc.tensor.matmul(
                            acc_[:, r0:512],
                            vxb_[:, j0 * 97:j0 * 97 + 97],
                            pt2_[:, r0:512], start=first, stop=False)
                        nc.tensor.matmul(
                            acc_[:, r1:512],
                            vxb_[:, (j0 + 1) * 97:(j0 + 1) * 97 + 97],
                            pt2_[:, 512 + r1:1024], start=False, stop=last_)
                    else:
                        j0 = 2 * jp_
                        nc.tensor.matmul(acc_[:],
                                         vxb_[:, j0 * 97:j0 * 97 + 97],
                                         pt2_[:, 0:512], start=first,
                                         stop=False)
                        nc.tensor.matmul(
                            acc_[:],
                            vxb_[:, (j0 + 1) * 97:(j0 + 1) * 97 + 97],
                            pt2_[:, 512:1024], start=False, stop=last_)

                norm_tail = [None]
                pending_rec = [None]

                def flush_norm(fill=False):
                    if norm_tail[0] is None:
                        return
                    b_, qh_, atus_, rrs_ = norm_tail[0]
                    norm_tail[0] = None
                    for ib_ in range(4):
                        bcpt = xps.tile([128, 512], F32, tag="x",
                                        name=f"bcp_{b_}_{qh_}_{ib_}")
                        bcp = bcpt[0:VD, :]
                        nc.tensor.matmul(bcp[:], ones96[:], rrs_[ib_][:],
                                         start=True, stop=True)
                        csl = slice(b_ * S + ib_ * 512,
                                    b_ * S + (ib_ + 1) * 512)
                        for (c, pa, pb, va, vb_) in AT_SEGS[qh_]:
                            nc.vector.tensor_mul(at_sb[pa:pb, c, csl],
                                                 atus_[ib_][va:vb_, :],
                                                 bcp[va:vb_, :])
                        if fill:       # keep PE fed between bcp groups
                            pop_citem()

                def flush_rec():
                    # previous qh's reciprocal chain, emitted once its
                    # denominator drains have cleared the DVE queue (one
                    # qh late) so it never blocks that qh's mask muls
                    if pending_rec[0] is None:
                        return
                    b_, qh_, dn4_, atus_ = pending_rec[0]
                    pending_rec[0] = None
                    rec4 = bpool.tile([97, 512], F32, tag="rec4",
                                      name=f"rec4_{b_}_{qh_}")
                    nc.vector.reciprocal_approx_fast(rec4[:], dn4_[:])
                    rrs = []
                    for i4 in range(4):
                        rr = bpool.tile([1, 512], F32R, tag="rr",
                                        bufs=8,
                                        name=f"rr_{b_}_{qh_}_{i4}")
                        nc.vector.tensor_copy(
                            rr[:], rec4[32 * i4:32 * i4 + 1, :])
                        rrs.append(rr)
                    norm_tail[0] = (b_, qh_, atus_, rrs)

                pends = []

                def pop_pv():
                    p = pends.pop(0)
                    emit_pv(p)
                    acc_, _, jp_, ib_, _, last_, b_, qh_, st = p
                    if not last_:
                        return
                    # drain this q tile: denominator row + bf16 cast
                    dn4_, atus_ = st["dn4"], st["atus"]
                    nc.vector.tensor_copy(
                        dn4_[32 * ib_:32 * ib_ + 1, :],
                        acc_[VD:VD + 1, :])
                    atu = bpool.tile([VD, 512], BF16, tag="atu", bufs=8,
                                     name=f"atu_{b_}_{qh_}_{ib_}")
                    nc.vector.tensor_copy(atu[:], acc_[0:VD, :])
                    atus_.append(atu)
                    if ib_ == 3:
                        pending_rec[0] = (b_, qh_, dn4_, atus_)

                for b in range(B):
                    ksb, vxb = ks[b], vx[b]
                    for qh in range(QH):
                        dn4 = bpool.tile([97, 512], F32, tag="dn4",
                                         name=f"dn4_{b}_{qh}")
                        st = {"dn4": dn4, "atus": []}
                        for ib in range(4):
                            qcols = qs[b][qh][:, ib * 512:(ib + 1) * 512]
                            acc = accb.tile([VD + 1, 512], F32, tag="acc",
                                            name=f"acc_{b}_{qh}_{ib}")
                            npair = 2 * ib + 2
                            for jp in range(npair):
                                stp2 = stps.tile([128, 1024], F32,
                                                 tag="stp")
                                pt2 = ptpool.tile([128, 1024], BF16,
                                                  tag="pt")
                                if jp >= 2 * ib:     # diagonal pair
                                    s0 = 2 * (jp - 2 * ib)
                                    r0, r1 = s0 * 128, s0 * 128 + 128
                                    j0 = 4 * ib + s0
                                    nc.tensor.matmul(
                                        stp2[:, r0:512],
                                        ksb[:, j0 * 128:(j0 + 1) * 128],
                                        qcols[:, r0:512], start=True,
                                        stop=True)
                                    nc.tensor.matmul(
                                        stp2[:, 512 + r1:1024],
                                        ksb[:, (j0 + 1) * 128:
                                            (j0 + 2) * 128],
                                        qcols[:, r1:512], start=True,
                                        stop=True)
                                    nc.scalar.activation(
                                        pt2[:, r0:512], stp2[:, r0:512],
                                        AFT.Exp, scale=SCALE)
                                    nc.scalar.activation(
                                        pt2[:, 512 + r1:1024],
                                        stp2[:, 512 + r1:1024],
                                        AFT.Exp, scale=SCALE)
                                    nc.vector.tensor_mul(
                                        pt2[:, r0:r0 + 128],
                                        pt2[:, r0:r0 + 128], msk_sb[:])
                                    nc.vector.tensor_mul(
                                        pt2[:, 512 + r1:512 + r1 + 128],
                                        pt2[:, 512 + r1:512 + r1 + 128],
                                        msk_sb[:])
                                else:                # two full blocks
                                    j0 = 2 * jp
                                    nc.tensor.matmul(
                                        stp2[:, 0:512],
                                        ksb[:, j0 * 128:(j0 + 1) * 128],
                                        qcols, start=True, stop=True)
                                    nc.tensor.matmul(
                                        stp2[:, 512:1024],
                                        ksb[:, (j0 + 1) * 128:
                                            (j0 + 2) * 128],
                                        qcols, start=True, stop=True)
                                    nc.scalar.activation(
                                        pt2[:], stp2[:], AFT.Exp,
                                        scale=SCALE)
                                if ib == 1 and jp == 0:
                                    flush_rec()
                                if ib == 2 and jp == 1:
                                    flush_norm(fill=(b == 1 and qh >= 1))
                                pends.append(
                                    (acc, pt2, jp, ib, vxb,
                                     jp == npair - 1, b, qh, st))
                                while len(pends) > 2:
                                    pop_pv()
                                if b == 1 and qh >= 1 and \
                                        len(citems) > 12:
                                    pop_citem()
                                    pop_citem()
                while pends:
                    pop_pv()
                flush_rec()               # b1 qh3 reciprocal chain
                for _ in range(4):        # cover the rec latency
                    pop_citem()
                flush_norm(fill=True)     # b1 qh3
                while citems:             # reserved b0 column groups
                    pop_citem()           # cover the B->C pool handover

            # ---------------- Phase C: batch-1 token columns -------------
            with (
                tc.tile_pool(name="opool2", bufs=2) as opool2,
                tc.tile_pool(name="tps", bufs=2, space="PSUM") as tps,
            ):
                for ct in range(NCT):
                    ops = [tps.tile([128, 512], F32, tag=f"c{tb}",
                                    name=f"ops{ct}_{tb}")
                           for tb in range(4)]
                    for oc in range(OSH // 128):
                        for tb in range(4):
                            nc.tensor.matmul(
                                ops[tb][:],
                                wo_sb[:, oc, ct * 128:(ct + 1) * 128],
                                at_sb[:, oc, S + tb * 512:S + (tb + 1) * 512],
                                start=(oc == 0), stop=(oc == OSH // 128 - 1))
                    osb = opool2.tile([128, S], BF16, tag="osb2",
                                      name=f"osb2_{ct}")
                    for tb in range(4):
                        tbs = slice(tb * 512, (tb + 1) * 512)
                        if tb % 2 == 0:
                            nc.vector.tensor_copy(osb[:, tbs], ops[tb][:])
                        else:
                            nc.scalar.copy(osb[:, tbs], ops[tb][:])
                    rsl = slice(ct * 128, (ct + 1) * 128)
                    nc.sync.dma_start(out=out[rsl, S:S + 1024],
                                      in_=osb[:, 0:1024])
                    nc.gpsimd.dma_start(out=out[rsl, S + 1024:S + 2048],
                                        in_=osb[:, 1024:2048])

    nc.compile()
    return nc


_NC_CACHE = None


def _get_nc():
    global _NC_CACHE
    if _NC_CACHE is None:
        _NC_CACHE = _build()
    return _NC_CACHE


def _host_tables(position_ids):
    pos = np.asarray(position_ids).reshape(-1)[:S].astype(np.float64)
    inv_freq = 1.0 / (THETA ** (np.arange(0, ROPE, 2, dtype=np.float64) / ROPE))
    freqs = np.outer(pos, inv_freq)                       # [S, ROPE/2]
    emb = np.concatenate([freqs, freqs], axis=-1)         # [S, ROPE]
    cos_t = np.tile(np.cos(emb).astype(np.float32).T, (1, B))  # [ROPE, T]
    sinf = np.sin(freqs).astype(np.float32).T                  # [ROPE/2, S]
    # rows 0:32 = +sin (t1[32:64] = q[0:32]*sin)
    # rows 32:64 = -sin (t1[0:32] = -q[32:64]*sin)
    sin_t = np.tile(np.concatenate([sinf, -sinf], axis=0), (1, B))
    return cos_t, sin_t


def kernel(hidden_states, attention_mask, position_ids, Wq, Wk, Wv, Wo,
           _trace=False, _tmpdir=None):
    import ml_dtypes
    from concourse.bass_utils import run_bass_kernel_spmd
    bf16 = ml_dtypes.bfloat16

    hidden = np.asarray(hidden_states, dtype=np.float32).reshape(T, HID)
    hiddent = np.ascontiguousarray(hidden.T).astype(bf16)
    Wq = np.asarray(Wq, dtype=np.float32)
    Wk = np.asarray(Wk, dtype=np.float32)
    Wv = np.asarray(Wv, dtype=np.float32)
    Wo = np.asarray(Wo, dtype=np.float32)
    cos_t, sin_t = _host_tables(position_ids)
    m = np.asarray(attention_mask).reshape(S, S)
    tri = np.ascontiguousarray((m[0:128, 0:128] == 0.0).T).astype(bf16)
    ident = np.eye(128, dtype=np.float32).astype(bf16)

    nc = _get_nc()
    in_maps = []
    for c in range(NCORES):
        in_maps.append({
            "hiddent": hiddent,
            "wq": np.ascontiguousarray(
                Wq[:, c * QH * HD:(c + 1) * QH * HD]).astype(bf16),
            "wk": np.ascontiguousarray(Wk[:, c * HD:(c + 1) * HD]).astype(bf16),
            "wv": np.ascontiguousarray(Wv[:, c * VD:(c + 1) * VD]).astype(bf16),
            "wo": np.ascontiguousarray(Wo[c * OSH:(c + 1) * OSH, :]).astype(bf16),
            "cos": cos_t, "sin": sin_t,
            "tri": tri, "ident": ident,
        })
    res = run_bass_kernel_spmd(nc, in_maps, list(range(NCORES)),
                               trace=_trace, tmpdir=_tmpdir)
    full = np.zeros((CO, T), dtype=np.float32)
    for c in range(NCORES):
        full += res.results[c]["out"].astype(np.float32)
    out = np.ascontiguousarray(full.T).reshape(B, S, CO)
    if _trace:
        kernel.last_exec_time_ns = res.exec_time_ns
        kernel.last_profile = res.profile_json
    return out
